# revision 25
# baseline (speedup 1.0000x reference)
"""Trainium2 Bass kernel for causal multi-head attention (dense transformer).

Problem shapes (hardcoded): x [2,2048,1024], 16 heads x 64 head-dim.
Sharding: data-parallel over batch (2) x tensor-parallel over heads (4/core)
on 8 NeuronCores. Each core computes the partial output (sum over its 4
heads) for one batch element; the host sums the 4 partials per batch and
adds b_O.

Per-core kernel, all fp16 on-chip (host pre-casts inputs, PSUM accumulates
fp32; fp16 streams the PE at 1 cycle/col with no narrow-matmul penalty):
  - host passes x^T and pre-transposed weights as fp16; DMA queues are laid
    out so WQ/WK and the first x^T chunks land first and WV/WO trail the
    x^T stream instead of stealing its bandwidth
  - phase 1a: QK projections for q 0:1024 plus V k-tiles 0..7, chunk-major
    so the PE starts on the first x^T chunk; zero-contribution "warm"
    matmuls pad the DMA-paced stretch so the PE HAM clock ramps to 2.4GHz
    and never gates down
  - phase 1b: the remaining QK/V projections run in 4 PSUM banks and are
    statically interleaved with the q-half-0 attention strips (4 banks), so
    the exp stream on ScalarE starts ~25us earlier and hides under PE work
  - Q/K biases ride the PSUM evacuation as per-partition tensor_scalar
    adds (no bias matmuls)
  - scores are computed as S^T[k,q] (k on partitions) with the contraction
    zero-padded from 64 to 128 rows; the causal mask is applied *in PSUM*
    by accumulating IDEN^T @ TRIM(-60) onto the diagonal block, so exp
    underflows to 0 in the fp16 output (no post-exp mask multiply)
  - exp is fused with the PSUM->SBUF evacuation on ScalarE (fp16 out)
  - AV uses V augmented with a ones column so the softmax denominator falls
    out of the same matmul
  - normalization per (head, q-half) right as the AV accumulation finishes:
    one denominator-row copy, reciprocal_approx_fast, gpsimd
    partition_broadcast, and a single fused tensor_tensor multiply that
    evacuates-and-normalizes z into fp16 ZN (h3/hf1 normalizes per 512-wide
    chunk so the final out-proj blocks unblock early)
  - output projection is interleaved into the q-half-1 attention stream as
    ZN q-blocks complete, borrowing score-strip PSUM tiles (stays within 8
    PSUM banks); partial outputs leave as fp16 (host sums in fp32)
"""

import sys

if "/opt/trn_rl_repo" not in sys.path:
    sys.path.insert(0, "/opt/trn_rl_repo")

import numpy as np

B, S, D = 2, 2048, 1024
H, DH = 16, 64
NCORES = 8
NH = 4            # heads per core
KCH = D // 128    # contraction chunks over model dim
NT = S // 128     # 128-row tiles over sequence
QC = S // 512     # 512-wide q chunks
P = 128
MASK_VAL = -60.0

_CACHE = {}


def _build_nc(debug=False):
    import concourse.tile as tile
    from concourse import bacc, mybir

    f32 = mybir.dt.float32
    f16 = mybir.dt.float16
    f8 = mybir.dt.float8e4
    DR = mybir.MatmulPerfMode.DoubleRow
    Exp = mybir.ActivationFunctionType.Exp
    mult = mybir.AluOpType.mult

    nc = bacc.Bacc("TRN2", target_bir_lowering=False, debug=False,
                   num_devices=NCORES)

    xt_d = nc.dram_tensor("xt", [D, S], f16, kind="ExternalInput").ap()
    wq_d = nc.dram_tensor("wq", [P, KCH * NH * DH], f16, kind="ExternalInput").ap()
    wk_d = nc.dram_tensor("wk", [P, KCH * NH * DH], f16, kind="ExternalInput").ap()
    wv_d = nc.dram_tensor("wv", [P, KCH * NH * DH], f16, kind="ExternalInput").ap()
    wo_d = nc.dram_tensor("wo", [P, 2 * D], f16, kind="ExternalInput").ap()
    bqc_d = nc.dram_tensor("bqc", [P, 2], f32, kind="ExternalInput").ap()
    bkc_d = nc.dram_tensor("bkc", [P, 2], f32, kind="ExternalInput").ap()
    bv_d = nc.dram_tensor("bv", [1, NH * DH], f16, kind="ExternalInput").ap()
    ones_d = nc.dram_tensor("ones", [1, S], f16, kind="ExternalInput").ap()
    vones_d = nc.dram_tensor("vones", [P, NT * NH], f16, kind="ExternalInput").ap()
    trim_d = nc.dram_tensor("trim", [P, P], f16, kind="ExternalInput").ap()
    iden_d = nc.dram_tensor("iden", [P, P], f16, kind="ExternalInput").ap()
    out_d = nc.dram_tensor("out", [S, D], f16, kind="ExternalOutput").ap()
    dbg = {}
    if debug:
        dbg["qt"] = nc.dram_tensor("dbg_qt", [P, 2 * S], f16, kind="ExternalOutput").ap()
        dbg["kt"] = nc.dram_tensor("dbg_kt", [P, NH * S], f16, kind="ExternalOutput").ap()
        dbg["v"] = nc.dram_tensor("dbg_v", [P, NT * NH * (DH + 1)], f16, kind="ExternalOutput").ap()
        dbg["zn"] = nc.dram_tensor("dbg_zn", [P, 2 * S], f16, kind="ExternalOutput").ap()

    with tile.TileContext(nc) as tc:
        from contextlib import ExitStack

        with ExitStack() as ctx:
            persist = ctx.enter_context(tc.tile_pool(name="persist", bufs=1))

            XT = persist.tile([P, KCH, S], f16)
            QT = persist.tile([P, 2, NH, S], f8)
            KT = persist.tile([P, 2, NH, S], f8)
            V = persist.tile([P, NT, NH, DH + 1], f16)
            ZN = persist.tile([P, 2, S], f16)
            WQ = persist.tile([P, KCH, NH * DH], f16)
            WK = persist.tile([P, KCH, NH * DH], f16)
            WV = persist.tile([P, KCH, NH * DH], f16)
            WO = persist.tile([P, 2, D], f16)
            BQC = persist.tile([P, 2], f32)
            BKC = persist.tile([P, 2], f32)
            BV = persist.tile([1, NH * DH], f16)
            ONES = persist.tile([1, S], f16)
            TRIM = persist.tile([P, P], f16)
            IDEN = persist.tile([P, P], f16)
            WARM = persist.tile([P, 512], f16)

            esp = ctx.enter_context(tc.tile_pool(name="esp", bufs=6))
            nrm = ctx.enter_context(tc.tile_pool(name="nrm", bufs=4))
            rrb = ctx.enter_context(tc.tile_pool(name="rrb", bufs=4))

            # ---- PE warmup on a memset tile (no DMA dependency) ----
            nc.vector.memset(WARM, 0.0)
            with tc.tile_pool(name="warm_ps", bufs=1, space="PSUM") as wp:
                wps = wp.tile([P, 512], f32)
                for i in range(10):
                    nc.tensor.matmul(wps, WARM[:, 0:P], WARM,
                                     start=True, stop=True)

            # preload the Exp activation table while DMAs stream
            with tc.tile_pool(name="pre", bufs=1) as pre:
                dumb = pre.tile([1, P], f16)
                nc.scalar.activation(dumb, WARM[0:1, 0:P], Exp)

            # zero the pad regions of the fp8 DoubleRow Q/K layout (rows
            # 64:128 of the data plane + the whole second k-tile plane) on
            # the otherwise-idle gpsimd; evacuations only touch rows 0:64 of
            # plane 0, so they proceed concurrently
            for T_ in (KT, QT):
                nc.gpsimd.memset(
                    T_[:, 1, :, :].rearrange("p a b -> p (a b)"), 0.0)
                nc.gpsimd.memset(
                    T_[64:128, 0, :, :].rearrange("p a b -> p (a b)"), 0.0)

            # ---- input DMAs (already fp16 on host) ----
            # gpsimd: small tensors + QK weights (arrive ~3us)
            nc.gpsimd.dma_start(WQ.rearrange("p a b -> p (a b)"), wq_d)
            nc.gpsimd.dma_start(BQC, bqc_d)
            nc.gpsimd.dma_start(BKC, bkc_d)
            nc.gpsimd.dma_start(BV, bv_d)
            nc.gpsimd.dma_start(ONES, ones_d)
            nc.gpsimd.dma_start(WK.rearrange("p a b -> p (a b)"), wk_d)
            # sync/scalar: x^T chunks; WV/vones/WO trail them (needed later)
            nc.scalar.dma_start(TRIM, trim_d)
            nc.scalar.dma_start(IDEN, iden_d)
            for ch in range(KCH):
                eng = nc.sync if ch % 2 == 0 else nc.scalar
                eng.dma_start(XT[:, ch, :],
                              xt_d[ch * P:(ch + 1) * P, :])
            nc.sync.dma_start(WV.rearrange("p a b -> p (a b)"), wv_d)
            nc.sync.dma_start(V[:, :, :, DH:DH + 1], vones_d)
            nc.scalar.dma_start(WO.rearrange("p a b -> p (a b)"), wo_d)

            def warm_fill(ps, n):
                # zero-contribution matmuls into a live PSUM tile: keeps the
                # PE streaming (HAM clock hot) while DMAs land
                for i in range(n):
                    nc.tensor.matmul(ps[:, 0:512], WARM[:, 0:P], WARM,
                                     start=False, stop=False,
                                     skip_group_check=True)

            def qk_evac(pst, wi, t, qc):
                ps = pst[(wi, t, qc)]
                sl = slice(qc * 512, (qc + 1) * 512)
                B_ = (BQC, BKC)[wi]
                T_ = (QT, KT)[wi]
                # bias rides the evacuation as a per-partition add; output
                # casts to the fp8 DoubleRow layout (k-tile 0, per head)
                nc.vector.tensor_scalar_add(
                    T_[0:64, 0, 2 * t, sl], ps[0:64, :], B_[0:64, t:t + 1])
                nc.vector.tensor_scalar_add(
                    T_[0:64, 0, 2 * t + 1, sl], ps[64:128, :],
                    B_[64:128, t:t + 1])

            # ---- phase 1a: QK sweep for q 0:1024 + V k-tiles 0..7 ----
            with tc.tile_pool(name="qkv_ps", bufs=8, space="PSUM") as qkv_ps:
                pst = {}
                for wi in range(2):
                    for t in range(2):
                        for qc in (0, 1):
                            pst[(wi, t, qc)] = qkv_ps.tile(
                                [P, 512], f32, tag="qk",
                                name=f"qk0_{wi}_{t}_{qc}")
                for ch in range(KCH):
                    for wi, W_ in enumerate((WQ, WK)):
                        for t in range(2):
                            for qc in (0, 1):
                                nc.tensor.matmul(
                                    pst[(wi, t, qc)],
                                    W_[:, ch, t * P:(t + 1) * P],
                                    XT[:, ch, qc * 512:(qc + 1) * 512],
                                    start=(ch == 0), stop=(ch == KCH - 1))
                    if ch < KCH - 1:
                        # absorb DMA pacing without letting the PE idle
                        warm_fill(pst[(0, 0, 0)], 6 if ch < 4 else 3)
                for wi in range(2):
                    for t in range(2):
                        for qc in (0, 1):
                            qk_evac(pst, wi, t, qc)
                # V k-tiles 0..7 (all x^T resident by now)
                psv = [qkv_ps.tile([P, 512], f32, tag="qk", name=f"v0_{i}")
                       for i in range(KCH)]
                for ch in range(KCH):
                    for i in range(KCH):
                        nc.tensor.matmul(
                            psv[i][:, 0:NH * DH],
                            XT[:, ch, i * P:(i + 1) * P],
                            WV[:, ch, :], start=(ch == 0), stop=False)
                for i in range(KCH):
                    nc.tensor.matmul(
                        psv[i][:, 0:NH * DH],
                        ONES[:, i * P:(i + 1) * P], BV,
                        start=False, stop=True)
                    # split V evacuations between ScalarE (idle here) and DVE
                    if i % 2 == 0:
                        nc.scalar.copy(
                            V[:, i, :, 0:DH], psv[i][:, 0:NH * DH])
                    else:
                        nc.vector.tensor_copy(
                            V[:, i, :, 0:DH], psv[i][:, 0:NH * DH])

            # ---- attention primitives ----
            # A "job" is a list of kb strips packed into ONE [128,1024] PSUM
            # tile (complementary causal widths), so a single ACTIVATE
            # evacuates them all.  h3/hf1 stays unpaired so its qc2 norm
            # lands early enough to unblock out-proj qt8..11.
            def _jobs(h, hf):
                if hf == 0:
                    return [[0], [1, 7], [2, 6], [3, 5], [4]]
                if h == NH - 1:
                    return [[kb] for kb in range(NT)]
                return ([[kb] for kb in range(9)]
                        + [[9, 15], [10, 14], [11, 13], [12]])

            def _last_done(h, hf):
                flat = [kb for j in _jobs(h, hf) for kb in j]
                m = {}
                for qc in ((0, 1) if hf == 0 else (2, 3)):
                    touch = [kb for kb in flat if kb <= 4 * qc + 3]
                    m.setdefault(touch[-1], set()).add(qc)
                return m

            def emit_scores(sc_ps, h, kbs, hf):
                t = h // 2
                hstart = hf * 1024
                strip_ps = sc_ps.tile([P, 1024], f32,
                                      name=f"sps_{h}_{kbs[0]}_{hf}",
                                      tag="sps")
                strip_sb = esp.tile([P, 1024], f16,
                                    name=f"ssb_{h}_{kbs[0]}_{hf}", tag="ssb")
                offs = {}
                off = 0
                for kb in kbs:
                    k0 = kb * P
                    qstart = max(k0, hstart)
                    offs[kb] = (off, qstart)
                    has_diag = k0 >= hstart
                    qpos = qstart
                    first = True
                    while qpos < hstart + 1024:
                        # chunk splits must fall on PSUM bank boundaries of
                        # the *tile* (col 512), which are shifted by `off`
                        # relative to the q coordinate
                        tcol = off + qpos - qstart
                        qnext = min(hstart + 1024, qpos + 512 - tcol % 512)
                        nc.tensor.matmul(
                            strip_ps[:, off + qpos - qstart:
                                     off + qnext - qstart],
                            KT[:, :, h, k0:k0 + P],
                            QT[:, :, h, qpos:qnext],
                            start=True, stop=not (has_diag and first),
                            perf_mode=DR)
                        if has_diag and first:
                            # accumulate IDEN^T @ TRIM = TRIM onto the
                            # diagonal block (piece-local column 0) so exp
                            # underflows to 0 above the diagonal
                            nc.tensor.matmul(
                                strip_ps[:, off:off + P], IDEN, TRIM,
                                start=False, stop=True,
                                skip_group_check=True)
                        first = False
                        qpos = qnext
                    off += hstart + 1024 - qstart
                nc.scalar.activation(
                    strip_sb[:, 0:off], strip_ps[:, 0:off], Exp)
                return strip_sb, offs

            avs = {}
            norm_cnt = {}
            drows = {}

            def _norm(h, hf, qcs_lo, width, src_ap, dst_sl):
                t, pb = h // 2, (h % 2) * 64
                drow = nrm.tile([1, width], f32, tag=f"dr{width}")
                nc.vector.tensor_copy(drow, qcs_lo)
                rr = nrm.tile([1, width], f32, tag=f"rr{width}")
                nc.vector.reciprocal_approx_fast(out=rr, in_=drow)
                rb = rrb.tile([64, width], f32, tag=f"rb{width}")
                nc.gpsimd.partition_broadcast(rb, rr)
                nc.vector.tensor_tensor(
                    ZN[pb:pb + 64, t, dst_sl], src_ap, rb, mult)

            def emit_av(av_ps, h, kbs, hf, strip_sb, offs):
                hstart = hf * 1024
                if kbs[0] == 0:
                    avs[(h, hf)] = av_ps.tile(
                        [DH + 1, 2, 512], f32,
                        tag="av", name=f"av_{h}_{hf}")
                    norm_cnt[(h, hf)] = 0
                av = avs[(h, hf)]
                last_done = _last_done(h, hf)
                for kb in kbs:
                    off, qstart = offs[kb]
                    qpos = qstart
                    while qpos < hstart + 1024:
                        qc = qpos // 512
                        qnext = min(hstart + 1024, (qc + 1) * 512)
                        done = qc in last_done.get(kb, ())
                        qr = qc - 2 * hf
                        nc.tensor.matmul(
                            av[:, qr, qpos - qc * 512:qnext - qc * 512],
                            V[:, kb, h, :],
                            strip_sb[:, off + qpos - qstart:
                                      off + qnext - qstart],
                            start=(kbs[0] == 0 and kb == 0), stop=done)
                        if done:
                            norm_cnt[(h, hf)] += 1
                            if h == NH - 1 and hf == 1:
                                # per-qc norm for the early out-proj unblock
                                _norm(h, hf,
                                      av[DH:DH + 1, qr, :], 512,
                                      av[0:DH, qr, :],
                                      slice(qc * 512, (qc + 1) * 512))
                            elif norm_cnt[(h, hf)] == 1:
                                # stage this half's denominator row early so
                                # only one copy sits on the final chain
                                dr = nrm.tile([1, 1024], f32, tag="dr2",
                                              name=f"dr_{h}_{hf}")
                                drows[(h, hf)] = dr
                                nc.vector.tensor_copy(
                                    dr[:, qr * 512:(qr + 1) * 512],
                                    av[DH:DH + 1, qr, :])
                            else:
                                dr = drows[(h, hf)]
                                nc.vector.tensor_copy(
                                    dr[:, qr * 512:(qr + 1) * 512],
                                    av[DH:DH + 1, qr, :])
                                t_, pb = h // 2, (h % 2) * 64
                                rr = nrm.tile([1, 1024], f32, tag="rr2")
                                nc.vector.reciprocal_approx_fast(
                                    out=rr, in_=dr)
                                rb = rrb.tile([64, 1024], f32, tag="rb2")
                                nc.gpsimd.partition_broadcast(rb, rr)
                                nc.vector.tensor_tensor(
                                    ZN[pb:pb + 64, t_,
                                       hstart:hstart + 1024],
                                    av[0:DH, :, :].rearrange(
                                        "p a b -> p (a b)"),
                                    rb, mult)
                        qpos = qnext

            def emit_op(sc_ps, osb, qt, act_evac=False, prewarm=0):
                # output projection for one 128-row q block, borrowing a
                # score-strip PSUM tile (keeps total PSUM at 8 banks)
                ps = sc_ps.tile([P, 1024], f32, tag="sps",
                                name=f"op_{qt}")
                if prewarm:
                    # dependency-free filler so the HAM clock stays hot while
                    # the last norm chain completes
                    for i in range(prewarm):
                        nc.tensor.matmul(ps[:, 0:512], WARM[:, 0:P], WARM,
                                         start=(i == 0),
                                         stop=(i == prewarm - 1),
                                         skip_group_check=True)
                for dc in range(2):
                    for t in range(2):
                        nc.tensor.matmul(
                            ps[:, dc * 512:(dc + 1) * 512],
                            ZN[:, t, qt * P:(qt + 1) * P],
                            WO[:, t, dc * 512:(dc + 1) * 512],
                            start=(t == 0), stop=(t == 1),
                            skip_group_check=bool(prewarm))
                ob = osb.tile([P, 1024], f16, tag="ob",
                              name=f"ob_{qt}")
                if act_evac:
                    # post-attention: ScalarE is idle, split the evac
                    nc.vector.tensor_copy(ob[:, 0:512], ps[:, 0:512])
                    nc.scalar.copy(ob[:, 512:1024], ps[:, 512:1024])
                else:
                    nc.vector.tensor_copy(ob, ps)
                oeng = (nc.sync, nc.scalar, nc.gpsimd)[qt % 3]
                oeng.dma_start(out_d[qt * P:(qt + 1) * P, :], ob)

            # ---- phase 1c: QK sweep for q 1024:2048 + V k-tiles 8..15 ----
            with tc.tile_pool(name="qkv_ps2", bufs=8, space="PSUM") as qkv2:
                pst = {}
                for wi in range(2):
                    for t in range(2):
                        for qc in (2, 3):
                            pst[(wi, t, qc)] = qkv2.tile(
                                [P, 512], f32, tag="qk",
                                name=f"qk1_{wi}_{t}_{qc}")
                for ch in range(KCH):
                    for wi, W_ in enumerate((WQ, WK)):
                        for t in range(2):
                            for qc in (2, 3):
                                nc.tensor.matmul(
                                    pst[(wi, t, qc)],
                                    W_[:, ch, t * P:(t + 1) * P],
                                    XT[:, ch, qc * 512:(qc + 1) * 512],
                                    start=(ch == 0), stop=(ch == KCH - 1))
                for wi in range(2):
                    for t in range(2):
                        for qc in (2, 3):
                            qk_evac(pst, wi, t, qc)
                psv = [qkv2.tile([P, 512], f32, tag="qk", name=f"v1_{i}")
                       for i in range(KCH)]
                for ch in range(KCH):
                    for i in range(KCH):
                        kt = KCH + i
                        nc.tensor.matmul(
                            psv[i][:, 0:NH * DH],
                            XT[:, ch, kt * P:(kt + 1) * P],
                            WV[:, ch, :], start=(ch == 0), stop=False)
                for i in range(KCH):
                    kt = KCH + i
                    nc.tensor.matmul(
                        psv[i][:, 0:NH * DH],
                        ONES[:, kt * P:(kt + 1) * P], BV,
                        start=False, stop=True)
                    if i % 2 == 0:
                        nc.scalar.copy(
                            V[:, kt, :, 0:DH], psv[i][:, 0:NH * DH])
                    else:
                        nc.vector.tensor_copy(
                            V[:, kt, :, 0:DH], psv[i][:, 0:NH * DH])

            # ---- phase 2: q-half-1 attention + interleaved out-proj ----
            from collections import deque

            def run_strips(sc_ps, av_ps, osb, work):
                pending = deque()
                for item in work:
                    if item[0] == "op":
                        emit_op(sc_ps, osb, item[1])
                        continue
                    sid = item[1]
                    sb_tile, offs = emit_scores(sc_ps, *sid)
                    pending.append((sid, sb_tile, offs))
                    if len(pending) > 2:
                        psid, psb, poffs = pending.popleft()
                        emit_av(av_ps, *psid, psb, poffs)
                while pending:
                    psid, psb, poffs = pending.popleft()
                    emit_av(av_ps, *psid, psb, poffs)

            work = []
            for h in range(NH):
                for j in _jobs(h, 0):
                    work.append(("sc", (h, j, 0)))
            for h in range(NH):
                cnt = 0
                for ji, j in enumerate(_jobs(h, 1)):
                    work.append(("sc", (h, j, 1)))
                    if h <= 1 and ji in (2, 5, 8, 11):
                        # qt0..7 (q first half) spread over h0/h1 groups
                        work.append(("op", 4 * h + cnt))
                        cnt += 1
                    # h3: the qc2 norm is emitted when the [11] job is
                    # popped (at the [13] push); qt8..11 need only
                    # ZN q 1024:1536
                    if h == NH - 1 and j[0] >= 13:
                        work.append(("op", j[0] - 5))
            work.append(("op", 11))

            with tc.tile_pool(name="sc_psA", bufs=2, space="PSUM") as scA, \
                    tc.tile_pool(name="av_psA", bufs=2, space="PSUM") as avA, \
                    tc.tile_pool(name="osb", bufs=3) as osb:
                run_strips(scA, avA, osb, work)
                if debug:
                    for nm, tl in (("qt", QT), ("kt", KT),
                                   ("v", V), ("zn", ZN)):
                        nc.gpsimd.dma_start(
                            dbg[nm], tl.rearrange("p ... -> p (...)"))
                # remaining out-projection (q 1536:2048); prewarm keeps the
                # clock hot across the final norm-chain wait
                for qt in range(12, NT):
                    emit_op(scA, osb, qt, act_evac=True,
                            prewarm=10 if qt == 12 else 0)

    nc.compile()
    return nc


def _get_nc(debug=False):
    key = ("nc", debug)
    if key not in _CACHE:
        _CACHE[key] = _build_nc(debug)
    return _CACHE[key]


def _host_inputs(x, W_Q, W_K, W_V, W_O, b_Q, b_K, b_V):
    """Build the 8 per-core input maps (all fp16)."""
    x = np.asarray(x, dtype=np.float32)
    scale = 1.0 / np.sqrt(np.float32(DH))
    ones = np.ones((1, S), dtype=np.float16)
    vones = np.ones((P, NT * NH), dtype=np.float16)
    trim = np.where(np.arange(P)[:, None] <= np.arange(P)[None, :],
                    np.float32(0.0), np.float32(MASK_VAL)).astype(np.float16)
    iden = np.eye(P, dtype=np.float16)

    xts = [np.ascontiguousarray(x[b].T).astype(np.float16) for b in range(B)]

    in_maps = []
    for c in range(NCORES):
        b, hg = divmod(c, NCORES // B)
        h0 = NH * hg
        def chunked(a):   # [D, M] -> [128, KCH*M] with rows p, cols (ch, m)
            return np.ascontiguousarray(
                a.reshape(KCH, P, -1).transpose(1, 0, 2).reshape(P, -1)
            ).astype(np.float16)
        hscale = np.float32(np.sqrt(scale))
        wq = chunked((np.asarray(W_Q[h0:h0 + NH], np.float32) * hscale)
                     .reshape(NH * DH, D).T)
        wk = chunked((np.asarray(W_K[h0:h0 + NH], np.float32) * hscale)
                     .reshape(NH * DH, D).T)
        wv = chunked(np.asarray(W_V[h0:h0 + NH], np.float32)
                     .reshape(NH * DH, D).T)
        wo_flat = np.asarray(W_O[h0:h0 + NH], np.float32) \
            .transpose(0, 2, 1).reshape(NH * DH, D)
        wo = np.ascontiguousarray(
            wo_flat.reshape(2, P, D).transpose(1, 0, 2).reshape(P, 2 * D)
        ).astype(np.float16)
        # per-partition bias columns: col t = heads (2t, 2t+1) x 64 dh
        bqc = np.ascontiguousarray(
            (np.asarray(b_Q[h0:h0 + NH], np.float32) * hscale)
            .reshape(2, P).T).astype(np.float32)
        bkc = np.ascontiguousarray(
            (np.asarray(b_K[h0:h0 + NH], np.float32) * hscale)
            .reshape(2, P).T).astype(np.float32)
        bv = np.asarray(b_V[h0:h0 + NH], np.float32) \
            .reshape(1, NH * DH).astype(np.float16)
        in_maps.append({
            "xt": xts[b], "wq": wq, "wk": wk, "wv": wv, "wo": wo,
            "bqc": bqc, "bkc": bkc,
            "bv": np.ascontiguousarray(bv), "ones": ones, "vones": vones,
            "trim": trim, "iden": iden,
        })
    return in_maps


def run_spmd(in_maps, debug=False, **kwargs):
    from concourse import bass_utils
    nc = _get_nc(debug)
    return bass_utils.run_bass_kernel_spmd(
        nc, in_maps, core_ids=list(range(NCORES)), **kwargs)


def kernel(x, W_Q, W_K, W_V, W_O, b_Q, b_K, b_V, b_O):
    in_maps = _host_inputs(x, W_Q, W_K, W_V, W_O, b_Q, b_K, b_V)
    res = run_spmd(in_maps)
    parts = [res.results[c]["out"].astype(np.float32) for c in range(NCORES)]
    gpb = NCORES // B
    out = np.stack(
        [sum(parts[b * gpb + g] for g in range(gpb)) for b in range(B)], axis=0)
    out += np.asarray(b_O, np.float32)[None, None, :]
    return out.astype(np.float32)


# revision 26
# speedup vs baseline: 1.3939x; 1.3939x over previous
"""Trainium2 Bass kernel for causal multi-head attention (dense transformer).

Problem shapes (hardcoded): x [2,2048,1024], 16 heads x 64 head-dim.
Sharding: data-parallel over batch (2) x tensor-parallel over heads (4/core)
on 8 NeuronCores. Each core computes the partial output (sum over its 4
heads) for one batch element; the host sums the 4 partials per batch and
adds b_O.

Per-core kernel, all fp16 on-chip (host pre-casts inputs, PSUM accumulates
fp32; fp16 streams the PE at 1 cycle/col with no narrow-matmul penalty):
  - host passes x^T and pre-transposed weights as fp16; DMA queues are laid
    out so WQ/WK and the first x^T chunks land first and WV/WO trail the
    x^T stream instead of stealing its bandwidth
  - phase 1a: QK projections for q 0:1024 plus V k-tiles 0..7, chunk-major
    so the PE starts on the first x^T chunk; zero-contribution "warm"
    matmuls pad the DMA-paced stretch so the PE HAM clock ramps to 2.4GHz
    and never gates down
  - phase 1b: the remaining QK/V projections run in 4 PSUM banks and are
    statically interleaved with the q-half-0 attention strips (4 banks), so
    the exp stream on ScalarE starts ~25us earlier and hides under PE work
  - Q/K biases ride the PSUM evacuation as per-partition tensor_scalar
    adds (no bias matmuls)
  - scores are computed as S^T[k,q] (k on partitions) with the contraction
    zero-padded from 64 to 128 rows; the causal mask is applied *in PSUM*
    by accumulating IDEN^T @ TRIM(-60) onto the diagonal block, so exp
    underflows to 0 in the fp16 output (no post-exp mask multiply)
  - exp is fused with the PSUM->SBUF evacuation on ScalarE (fp16 out)
  - AV uses V augmented with a ones column so the softmax denominator falls
    out of the same matmul
  - normalization per (head, q-half) right as the AV accumulation finishes:
    one denominator-row copy, reciprocal_approx_fast, gpsimd
    partition_broadcast, and a single fused tensor_tensor multiply that
    evacuates-and-normalizes z into fp16 ZN (h3/hf1 normalizes per 512-wide
    chunk so the final out-proj blocks unblock early)
  - output projection is interleaved into the q-half-1 attention stream as
    ZN q-blocks complete, borrowing score-strip PSUM tiles (stays within 8
    PSUM banks); partial outputs leave as fp16 (host sums in fp32)
"""

import sys

if "/opt/trn_rl_repo" not in sys.path:
    sys.path.insert(0, "/opt/trn_rl_repo")

import numpy as np

B, S, D = 2, 2048, 1024
H, DH = 16, 64
NCORES = 8
NH = 4            # heads per core
KCH = D // 128    # contraction chunks over model dim
NT = S // 128     # 128-row tiles over sequence
QC = S // 512     # 512-wide q chunks
P = 128
MASK_VAL = -60.0

_CACHE = {}


def _build_nc(debug=False):
    import concourse.tile as tile
    from concourse import bacc, mybir

    f32 = mybir.dt.float32
    f16 = mybir.dt.float16
    Exp = mybir.ActivationFunctionType.Exp
    mult = mybir.AluOpType.mult

    nc = bacc.Bacc("TRN2", target_bir_lowering=False, debug=False,
                   num_devices=NCORES)

    xt_d = nc.dram_tensor("xt", [D, S], f16, kind="ExternalInput").ap()
    wq_d = nc.dram_tensor("wq", [P, KCH * NH * DH], f16, kind="ExternalInput").ap()
    wk_d = nc.dram_tensor("wk", [P, KCH * NH * DH], f16, kind="ExternalInput").ap()
    wv_d = nc.dram_tensor("wv", [P, KCH * NH * DH], f16, kind="ExternalInput").ap()
    wo_d = nc.dram_tensor("wo", [P, 2 * D], f16, kind="ExternalInput").ap()
    bqc_d = nc.dram_tensor("bqc", [P, 2], f32, kind="ExternalInput").ap()
    bkc_d = nc.dram_tensor("bkc", [P, 2], f32, kind="ExternalInput").ap()
    bv_d = nc.dram_tensor("bv", [1, NH * DH], f16, kind="ExternalInput").ap()
    ones_d = nc.dram_tensor("ones", [1, S], f16, kind="ExternalInput").ap()
    vones_d = nc.dram_tensor("vones", [P, NT * NH], f16, kind="ExternalInput").ap()
    trim_d = nc.dram_tensor("trim", [P, P], f16, kind="ExternalInput").ap()
    iden_d = nc.dram_tensor("iden", [P, P], f16, kind="ExternalInput").ap()
    out_d = nc.dram_tensor("out", [S, D], f16, kind="ExternalOutput").ap()
    dbg = {}
    if debug:
        dbg["qt"] = nc.dram_tensor("dbg_qt", [P, 2 * S], f16, kind="ExternalOutput").ap()
        dbg["kt"] = nc.dram_tensor("dbg_kt", [P, NH * S], f16, kind="ExternalOutput").ap()
        dbg["v"] = nc.dram_tensor("dbg_v", [P, NT * NH * (DH + 1)], f16, kind="ExternalOutput").ap()
        dbg["zn"] = nc.dram_tensor("dbg_zn", [P, 2 * S], f16, kind="ExternalOutput").ap()

    with tile.TileContext(nc) as tc:
        from contextlib import ExitStack

        with ExitStack() as ctx:
            persist = ctx.enter_context(tc.tile_pool(name="persist", bufs=1))

            XT = persist.tile([P, KCH, S], f16)
            QT = persist.tile([P, 2, S], f16)
            KT = persist.tile([P, NH, S], f16)
            V = persist.tile([P, NT, NH, DH + 1], f16)
            ZN = persist.tile([P, 2, S], f16)
            WQ = persist.tile([P, KCH, NH * DH], f16)
            WK = persist.tile([P, KCH, NH * DH], f16)
            WV = persist.tile([P, KCH, NH * DH], f16)
            WO = persist.tile([P, 2, D], f16)
            BQC = persist.tile([P, 2], f32)
            BKC = persist.tile([P, 2], f32)
            BV = persist.tile([1, NH * DH], f16)
            ONES = persist.tile([1, S], f16)
            TRIM = persist.tile([P, P], f16)
            IDEN = persist.tile([P, P], f16)
            WARM = persist.tile([P, 512], f16)

            esp = ctx.enter_context(tc.tile_pool(name="esp", bufs=6))
            nrm = ctx.enter_context(tc.tile_pool(name="nrm", bufs=4))
            rrb = ctx.enter_context(tc.tile_pool(name="rrb", bufs=4))

            # ---- PE warmup on a memset tile (no DMA dependency) ----
            nc.vector.memset(WARM, 0.0)
            with tc.tile_pool(name="warm_ps", bufs=1, space="PSUM") as wp:
                wps = wp.tile([P, 512], f32)
                for i in range(10):
                    nc.tensor.matmul(wps, WARM[:, 0:P], WARM,
                                     start=True, stop=True)

            # preload the Exp activation table while DMAs stream
            with tc.tile_pool(name="pre", bufs=1) as pre:
                dumb = pre.tile([1, P], f16)
                nc.scalar.activation(dumb, WARM[0:1, 0:P], Exp)

            # zero the pad half of KT (head h occupies partitions
            # (h%2)*64 .. +64 of column-block h; the rest must be 0)
            nc.vector.memset(KT.rearrange("p a b -> p (a b)"), 0.0)

            # ---- input DMAs (already fp16 on host) ----
            # gpsimd: small tensors + QK weights (arrive ~3us)
            nc.gpsimd.dma_start(WQ.rearrange("p a b -> p (a b)"), wq_d)
            nc.gpsimd.dma_start(BQC, bqc_d)
            nc.gpsimd.dma_start(BKC, bkc_d)
            nc.gpsimd.dma_start(BV, bv_d)
            nc.gpsimd.dma_start(ONES, ones_d)
            nc.gpsimd.dma_start(WK.rearrange("p a b -> p (a b)"), wk_d)
            # sync/scalar: x^T chunks; WV/vones/WO trail them (needed later)
            nc.scalar.dma_start(TRIM, trim_d)
            nc.scalar.dma_start(IDEN, iden_d)
            for ch in range(KCH):
                eng = nc.sync if ch % 2 == 0 else nc.scalar
                eng.dma_start(XT[:, ch, :],
                              xt_d[ch * P:(ch + 1) * P, :])
            nc.sync.dma_start(WV.rearrange("p a b -> p (a b)"), wv_d)
            nc.sync.dma_start(V[:, :, :, DH:DH + 1], vones_d)
            nc.scalar.dma_start(WO.rearrange("p a b -> p (a b)"), wo_d)

            def warm_fill(ps, n):
                # zero-contribution matmuls into a live PSUM tile: keeps the
                # PE streaming (HAM clock hot) while DMAs land
                for i in range(n):
                    nc.tensor.matmul(ps[:, 0:512], WARM[:, 0:P], WARM,
                                     start=False, stop=False,
                                     skip_group_check=True)

            def qk_evac(pst, wi, t, qc):
                ps = pst[(wi, t, qc)]
                sl = slice(qc * 512, (qc + 1) * 512)
                B_ = (BQC, BKC)[wi]
                # bias rides the evacuation as a per-partition add
                if wi == 0:
                    nc.vector.tensor_scalar_add(
                        QT[:, t, sl], ps, B_[:, t:t + 1])
                else:
                    nc.vector.tensor_scalar_add(
                        KT[0:64, 2 * t, sl], ps[0:64, :],
                        B_[0:64, t:t + 1])
                    nc.vector.tensor_scalar_add(
                        KT[64:128, 2 * t + 1, sl], ps[64:128, :],
                        B_[64:128, t:t + 1])

            # ---- phase 1a: QK sweep for q 0:1024 + V k-tiles 0..7 ----
            with tc.tile_pool(name="qkv_ps", bufs=8, space="PSUM") as qkv_ps:
                pst = {}
                for wi in range(2):
                    for t in range(2):
                        for qc in (0, 1):
                            pst[(wi, t, qc)] = qkv_ps.tile(
                                [P, 512], f32, tag="qk",
                                name=f"qk0_{wi}_{t}_{qc}")
                for ch in range(KCH):
                    for wi, W_ in enumerate((WQ, WK)):
                        for t in range(2):
                            for qc in (0, 1):
                                nc.tensor.matmul(
                                    pst[(wi, t, qc)],
                                    W_[:, ch, t * P:(t + 1) * P],
                                    XT[:, ch, qc * 512:(qc + 1) * 512],
                                    start=(ch == 0), stop=(ch == KCH - 1))
                    if ch < KCH - 1:
                        # absorb DMA pacing without letting the PE idle
                        warm_fill(pst[(0, 0, 0)], 6 if ch < 4 else 3)
                for wi in range(2):
                    for t in range(2):
                        for qc in (0, 1):
                            qk_evac(pst, wi, t, qc)
                # V k-tiles 0..7 (all x^T resident by now)
                psv = [qkv_ps.tile([P, 512], f32, tag="qk", name=f"v0_{i}")
                       for i in range(KCH)]
                for ch in range(KCH):
                    for i in range(KCH):
                        nc.tensor.matmul(
                            psv[i][:, 0:NH * DH],
                            XT[:, ch, i * P:(i + 1) * P],
                            WV[:, ch, :], start=(ch == 0), stop=False)
                for i in range(KCH):
                    nc.tensor.matmul(
                        psv[i][:, 0:NH * DH],
                        ONES[:, i * P:(i + 1) * P], BV,
                        start=False, stop=True)
                    # split V evacuations between ScalarE (idle here) and DVE
                    if i % 2 == 0:
                        nc.scalar.copy(
                            V[:, i, :, 0:DH], psv[i][:, 0:NH * DH])
                    else:
                        nc.vector.tensor_copy(
                            V[:, i, :, 0:DH], psv[i][:, 0:NH * DH])

            # ---- attention primitives ----
            # A "job" is a list of kb strips packed into ONE [128,1024] PSUM
            # tile (complementary causal widths), so a single ACTIVATE
            # evacuates them all.  h3/hf1 stays unpaired so its qc2 norm
            # lands early enough to unblock out-proj qt8..11.
            def _jobs(h, hf):
                if hf == 0:
                    return [[0], [1, 7], [2, 6], [3, 5], [4]]
                if h == NH - 1:
                    return [[kb] for kb in range(NT)]
                return ([[kb] for kb in range(9)]
                        + [[9, 15], [10, 14], [11, 13], [12]])

            def _last_done(h, hf):
                flat = [kb for j in _jobs(h, hf) for kb in j]
                m = {}
                for qc in ((0, 1) if hf == 0 else (2, 3)):
                    touch = [kb for kb in flat if kb <= 4 * qc + 3]
                    m.setdefault(touch[-1], set()).add(qc)
                return m

            def emit_scores(sc_ps, h, kbs, hf):
                t = h // 2
                hstart = hf * 1024
                strip_ps = sc_ps.tile([P, 1024], f32,
                                      name=f"sps_{h}_{kbs[0]}_{hf}",
                                      tag="sps")
                strip_sb = esp.tile([P, 1024], f16,
                                    name=f"ssb_{h}_{kbs[0]}_{hf}", tag="ssb")
                offs = {}
                off = 0
                for kb in kbs:
                    k0 = kb * P
                    qstart = max(k0, hstart)
                    offs[kb] = (off, qstart)
                    has_diag = k0 >= hstart
                    qpos = qstart
                    first = True
                    while qpos < hstart + 1024:
                        # chunk splits must fall on PSUM bank boundaries of
                        # the *tile* (col 512), which are shifted by `off`
                        # relative to the q coordinate
                        tcol = off + qpos - qstart
                        qnext = min(hstart + 1024, qpos + 512 - tcol % 512)
                        nc.tensor.matmul(
                            strip_ps[:, off + qpos - qstart:
                                     off + qnext - qstart],
                            KT[:, h, k0:k0 + P],
                            QT[:, t, qpos:qnext],
                            start=True, stop=not (has_diag and first))
                        if has_diag and first:
                            # accumulate IDEN^T @ TRIM = TRIM onto the
                            # diagonal block (piece-local column 0) so exp
                            # underflows to 0 above the diagonal
                            nc.tensor.matmul(
                                strip_ps[:, off:off + P], IDEN, TRIM,
                                start=False, stop=True,
                                skip_group_check=True)
                        first = False
                        qpos = qnext
                    off += hstart + 1024 - qstart
                nc.scalar.activation(
                    strip_sb[:, 0:off], strip_ps[:, 0:off], Exp)
                return strip_sb, offs

            avs = {}
            norm_cnt = {}
            drows = {}

            def _norm(h, hf, qcs_lo, width, src_ap, dst_sl):
                t, pb = h // 2, (h % 2) * 64
                drow = nrm.tile([1, width], f32, tag=f"dr{width}")
                nc.vector.tensor_copy(drow, qcs_lo)
                rr = nrm.tile([1, width], f32, tag=f"rr{width}")
                nc.vector.reciprocal_approx_fast(out=rr, in_=drow)
                rb = rrb.tile([64, width], f32, tag=f"rb{width}")
                nc.gpsimd.partition_broadcast(rb, rr)
                nc.vector.tensor_tensor(
                    ZN[pb:pb + 64, t, dst_sl], src_ap, rb, mult)

            def emit_av(av_ps, h, kbs, hf, strip_sb, offs):
                hstart = hf * 1024
                if kbs[0] == 0:
                    avs[(h, hf)] = av_ps.tile(
                        [DH + 1, 2, 512], f32,
                        tag="av", name=f"av_{h}_{hf}")
                    norm_cnt[(h, hf)] = 0
                av = avs[(h, hf)]
                last_done = _last_done(h, hf)
                for kb in kbs:
                    off, qstart = offs[kb]
                    qpos = qstart
                    while qpos < hstart + 1024:
                        qc = qpos // 512
                        qnext = min(hstart + 1024, (qc + 1) * 512)
                        done = qc in last_done.get(kb, ())
                        qr = qc - 2 * hf
                        nc.tensor.matmul(
                            av[:, qr, qpos - qc * 512:qnext - qc * 512],
                            V[:, kb, h, :],
                            strip_sb[:, off + qpos - qstart:
                                      off + qnext - qstart],
                            start=(kbs[0] == 0 and kb == 0), stop=done)
                        if done:
                            norm_cnt[(h, hf)] += 1
                            if h == NH - 1 and hf == 1:
                                # per-qc norm for the early out-proj unblock
                                _norm(h, hf,
                                      av[DH:DH + 1, qr, :], 512,
                                      av[0:DH, qr, :],
                                      slice(qc * 512, (qc + 1) * 512))
                            elif norm_cnt[(h, hf)] == 1:
                                # stage this half's denominator row early so
                                # only one copy sits on the final chain
                                dr = nrm.tile([1, 1024], f32, tag="dr2",
                                              name=f"dr_{h}_{hf}")
                                drows[(h, hf)] = dr
                                nc.vector.tensor_copy(
                                    dr[:, qr * 512:(qr + 1) * 512],
                                    av[DH:DH + 1, qr, :])
                            else:
                                dr = drows[(h, hf)]
                                nc.vector.tensor_copy(
                                    dr[:, qr * 512:(qr + 1) * 512],
                                    av[DH:DH + 1, qr, :])
                                t_, pb = h // 2, (h % 2) * 64
                                rr = nrm.tile([1, 1024], f32, tag="rr2")
                                nc.vector.reciprocal_approx_fast(
                                    out=rr, in_=dr)
                                rb = rrb.tile([64, 1024], f32, tag="rb2")
                                nc.gpsimd.partition_broadcast(rb, rr)
                                nc.vector.tensor_tensor(
                                    ZN[pb:pb + 64, t_,
                                       hstart:hstart + 1024],
                                    av[0:DH, :, :].rearrange(
                                        "p a b -> p (a b)"),
                                    rb, mult)
                        qpos = qnext

            def emit_op(sc_ps, osb, qt, act_evac=False, prewarm=0):
                # output projection for one 128-row q block, borrowing a
                # score-strip PSUM tile (keeps total PSUM at 8 banks)
                ps = sc_ps.tile([P, 1024], f32, tag="sps",
                                name=f"op_{qt}")
                if prewarm:
                    # dependency-free filler so the HAM clock stays hot while
                    # the last norm chain completes
                    for i in range(prewarm):
                        nc.tensor.matmul(ps[:, 0:512], WARM[:, 0:P], WARM,
                                         start=(i == 0),
                                         stop=(i == prewarm - 1),
                                         skip_group_check=True)
                for dc in range(2):
                    for t in range(2):
                        nc.tensor.matmul(
                            ps[:, dc * 512:(dc + 1) * 512],
                            ZN[:, t, qt * P:(qt + 1) * P],
                            WO[:, t, dc * 512:(dc + 1) * 512],
                            start=(t == 0), stop=(t == 1),
                            skip_group_check=bool(prewarm))
                ob = osb.tile([P, 1024], f16, tag="ob",
                              name=f"ob_{qt}")
                if act_evac:
                    # post-attention: ScalarE is idle, split the evac
                    nc.vector.tensor_copy(ob[:, 0:512], ps[:, 0:512])
                    nc.scalar.copy(ob[:, 512:1024], ps[:, 512:1024])
                else:
                    nc.vector.tensor_copy(ob, ps)
                oeng = (nc.sync, nc.scalar, nc.gpsimd)[qt % 3]
                oeng.dma_start(out_d[qt * P:(qt + 1) * P, :], ob)

            # ---- phase 1c: QK sweep for q 1024:2048 + V k-tiles 8..15 ----
            with tc.tile_pool(name="qkv_ps2", bufs=8, space="PSUM") as qkv2:
                pst = {}
                for wi in range(2):
                    for t in range(2):
                        for qc in (2, 3):
                            pst[(wi, t, qc)] = qkv2.tile(
                                [P, 512], f32, tag="qk",
                                name=f"qk1_{wi}_{t}_{qc}")
                for ch in range(KCH):
                    for wi, W_ in enumerate((WQ, WK)):
                        for t in range(2):
                            for qc in (2, 3):
                                nc.tensor.matmul(
                                    pst[(wi, t, qc)],
                                    W_[:, ch, t * P:(t + 1) * P],
                                    XT[:, ch, qc * 512:(qc + 1) * 512],
                                    start=(ch == 0), stop=(ch == KCH - 1))
                for wi in range(2):
                    for t in range(2):
                        for qc in (2, 3):
                            qk_evac(pst, wi, t, qc)
                psv = [qkv2.tile([P, 512], f32, tag="qk", name=f"v1_{i}")
                       for i in range(KCH)]
                for ch in range(KCH):
                    for i in range(KCH):
                        kt = KCH + i
                        nc.tensor.matmul(
                            psv[i][:, 0:NH * DH],
                            XT[:, ch, kt * P:(kt + 1) * P],
                            WV[:, ch, :], start=(ch == 0), stop=False)
                for i in range(KCH):
                    kt = KCH + i
                    nc.tensor.matmul(
                        psv[i][:, 0:NH * DH],
                        ONES[:, kt * P:(kt + 1) * P], BV,
                        start=False, stop=True)
                    if i % 2 == 0:
                        nc.scalar.copy(
                            V[:, kt, :, 0:DH], psv[i][:, 0:NH * DH])
                    else:
                        nc.vector.tensor_copy(
                            V[:, kt, :, 0:DH], psv[i][:, 0:NH * DH])

            # ---- phase 2: q-half-1 attention + interleaved out-proj ----
            from collections import deque

            def run_strips(sc_ps, av_ps, osb, work):
                pending = deque()
                for item in work:
                    if item[0] == "op":
                        emit_op(sc_ps, osb, item[1])
                        continue
                    sid = item[1]
                    sb_tile, offs = emit_scores(sc_ps, *sid)
                    pending.append((sid, sb_tile, offs))
                    if len(pending) > 2:
                        psid, psb, poffs = pending.popleft()
                        emit_av(av_ps, *psid, psb, poffs)
                while pending:
                    psid, psb, poffs = pending.popleft()
                    emit_av(av_ps, *psid, psb, poffs)

            work = []
            for h in range(NH):
                for j in _jobs(h, 0):
                    work.append(("sc", (h, j, 0)))
            for h in range(NH):
                cnt = 0
                for ji, j in enumerate(_jobs(h, 1)):
                    work.append(("sc", (h, j, 1)))
                    if h <= 1 and ji in (2, 5, 8, 11):
                        # qt0..7 (q first half) spread over h0/h1 groups
                        work.append(("op", 4 * h + cnt))
                        cnt += 1
                    # h3: the qc2 norm is emitted when the [11] job is
                    # popped (at the [13] push); qt8..11 need only
                    # ZN q 1024:1536
                    if h == NH - 1 and j[0] >= 13:
                        work.append(("op", j[0] - 5))
            work.append(("op", 11))

            with tc.tile_pool(name="sc_psA", bufs=2, space="PSUM") as scA, \
                    tc.tile_pool(name="av_psA", bufs=2, space="PSUM") as avA, \
                    tc.tile_pool(name="osb", bufs=3) as osb:
                run_strips(scA, avA, osb, work)
                if debug:
                    for nm, tl in (("qt", QT), ("kt", KT),
                                   ("v", V), ("zn", ZN)):
                        nc.gpsimd.dma_start(
                            dbg[nm], tl.rearrange("p ... -> p (...)"))
                # remaining out-projection (q 1536:2048); prewarm keeps the
                # clock hot across the final norm-chain wait
                for qt in range(12, NT):
                    emit_op(scA, osb, qt, act_evac=True,
                            prewarm=10 if qt == 12 else 0)

    nc.compile()
    return nc


def _get_nc(debug=False):
    key = ("nc", debug)
    if key not in _CACHE:
        _CACHE[key] = _build_nc(debug)
    return _CACHE[key]


def _host_inputs(x, W_Q, W_K, W_V, W_O, b_Q, b_K, b_V):
    """Build the 8 per-core input maps (all fp16)."""
    x = np.asarray(x, dtype=np.float32)
    scale = 1.0 / np.sqrt(np.float32(DH))
    ones = np.ones((1, S), dtype=np.float16)
    vones = np.ones((P, NT * NH), dtype=np.float16)
    trim = np.where(np.arange(P)[:, None] <= np.arange(P)[None, :],
                    np.float32(0.0), np.float32(MASK_VAL)).astype(np.float16)
    iden = np.eye(P, dtype=np.float16)

    xts = [np.ascontiguousarray(x[b].T).astype(np.float16) for b in range(B)]

    in_maps = []
    for c in range(NCORES):
        b, hg = divmod(c, NCORES // B)
        h0 = NH * hg
        def chunked(a):   # [D, M] -> [128, KCH*M] with rows p, cols (ch, m)
            return np.ascontiguousarray(
                a.reshape(KCH, P, -1).transpose(1, 0, 2).reshape(P, -1)
            ).astype(np.float16)
        wq = chunked((np.asarray(W_Q[h0:h0 + NH], np.float32) * scale)
                     .reshape(NH * DH, D).T)
        wk = chunked(np.asarray(W_K[h0:h0 + NH], np.float32)
                     .reshape(NH * DH, D).T)
        wv = chunked(np.asarray(W_V[h0:h0 + NH], np.float32)
                     .reshape(NH * DH, D).T)
        wo_flat = np.asarray(W_O[h0:h0 + NH], np.float32) \
            .transpose(0, 2, 1).reshape(NH * DH, D)
        wo = np.ascontiguousarray(
            wo_flat.reshape(2, P, D).transpose(1, 0, 2).reshape(P, 2 * D)
        ).astype(np.float16)
        # per-partition bias columns: col t = heads (2t, 2t+1) x 64 dh
        bqc = np.ascontiguousarray(
            (np.asarray(b_Q[h0:h0 + NH], np.float32) * scale)
            .reshape(2, P).T).astype(np.float32)
        bkc = np.ascontiguousarray(
            np.asarray(b_K[h0:h0 + NH], np.float32)
            .reshape(2, P).T).astype(np.float32)
        bv = np.asarray(b_V[h0:h0 + NH], np.float32) \
            .reshape(1, NH * DH).astype(np.float16)
        in_maps.append({
            "xt": xts[b], "wq": wq, "wk": wk, "wv": wv, "wo": wo,
            "bqc": bqc, "bkc": bkc,
            "bv": np.ascontiguousarray(bv), "ones": ones, "vones": vones,
            "trim": trim, "iden": iden,
        })
    return in_maps


def run_spmd(in_maps, debug=False, **kwargs):
    from concourse import bass_utils
    nc = _get_nc(debug)
    return bass_utils.run_bass_kernel_spmd(
        nc, in_maps, core_ids=list(range(NCORES)), **kwargs)


def kernel(x, W_Q, W_K, W_V, W_O, b_Q, b_K, b_V, b_O):
    in_maps = _host_inputs(x, W_Q, W_K, W_V, W_O, b_Q, b_K, b_V)
    res = run_spmd(in_maps)
    parts = [res.results[c]["out"].astype(np.float32) for c in range(NCORES)]
    gpb = NCORES // B
    out = np.stack(
        [sum(parts[b * gpb + g] for g in range(gpb)) for b in range(B)], axis=0)
    out += np.asarray(b_O, np.float32)[None, None, :]
    return out.astype(np.float32)
